# revision 1
# baseline (speedup 1.0000x reference)
"""Bass/Tile TRN2 kernel for nn_BernoulliMaskedPPCA (loss_fn).

Math (see reference): with m = int(0.15*D) = 117 masked dims from the LAST
permutation only,
    logits = Wm @ z_int.T + bm[:, None]                  (m, L^2)
    log_prob_x = xm @ log_p1 + (1-xm) @ log_p0           (N, L^2)
               = xm @ logits + sum_j log_p0[j, :]         (x is binary)
    loss = -(D / (P*m*N)) * sum_n logsumexp_c(log_w + log_p_z + log_prob_x)

Strategy (data-parallel, per sharding hint):
  - Host: gather xm = x[:, perm[:m]], transpose to (m+1, N) with a ones row
    appended (folds the per-column constant c_row into the GEMM), cast to
    bf16 (exact for binary x), shard along N across 8 cores.
  - Host: LdAug = [logits; c_row] (118 x 400) built in float64, split into
    bf16 hi + lo so 2 accumulating PE matmuls reproduce fp32 accuracy
    (~7e-7 rel err on the final scalar, validated offline).
  - Device per core: 64 row-tiles of 128; per tile 2 bf16 matmuls into one
    PSUM bank; strided DVE submax (negate=True) batched over 4 banks gives
    the exp shift; ScalarE Exp in place in PSUM with per-partition bias;
    row-sums split between the ACT accumulator and batched DVE reduces.
  - Device outputs per core: S (sum of exps) and -shift, each (128, 64) f32.
  - Host: lse = ln(S) + shift summed in float64, scaled, returned as f32.
"""

import numpy as np
import ml_dtypes

import concourse.bacc as bacc
import concourse.tile as tile
import concourse.mybir as mybir
from concourse.bass_utils import run_bass_kernel_spmd

N_CORES = 8
N_OBS = 65536
D_DIM = 784
M_DIM = 117  # int(784 * 0.15)
K_DIM = M_DIM + 1  # + ones row for the c_row constant
L_BINS = 20
L2 = L_BINS * L_BINS  # 400
N_PERM = 4
ROWS_PER_CORE = N_OBS // N_CORES  # 8192
PART = 128
N_TILES = ROWS_PER_CORE // PART  # 64
BGRP = 2  # PSUM banks per group (bufs=4 -> 4 groups in flight)
N_GRPS = N_TILES // BGRP  # 32

_COMPILED = None
LAST_RESULTS = None


def _emit_compute(nc, tc, stats, psum, xmt_d, xmt_sb, ldhi_sb, ldlo_sb,
                  negm_sb, s_sb, do_dve=True, do_act=True, act_accum=True):
    """One full pass: DMA the x shard in, GEMM + shifted-exp row sums.

    do_dve/do_act/act_accum are benchmark-only ablation switches
    (numerically wrong when False; used to attribute HW time per engine)."""
    # Fine-grained chunks spread over the HWDGE queues: the first tile's
    # operand lands ~4x sooner than with 2048-col chunks, so PE starts
    # earlier (the chunk-0 wait is serial time at every kernel start).
    chunk = 512
    for k in range(ROWS_PER_CORE // chunk):
        sl = slice(k * chunk, (k + 1) * chunk)
        nc.sync.dma_start(out=xmt_sb[:, sl], in_=xmt_d[:, sl])

    # Prime the exp activation table while input DMAs run, so the ~1.3us
    # table load is off the critical path.
    if do_act:
        prime = stats.tile([PART, 1], mybir.dt.float32, tag="prime")
        nc.vector.memset(prime, 0.0)
        nc.scalar.activation(
            out=prime, in_=prime, func=mybir.ActivationFunctionType.Exp
        )

    # The exp shift need not be the exact row max: any per-row value within
    # ~80 of it avoids fp32 overflow/underflow, and the shift is added back
    # exactly, so correctness is shift-independent. A strided submax (every
    # 4th grid column, offset 2) is within ~11 of the true max on this
    # problem's data (validated offline, with large margin even under
    # re-randomized inputs), and costs 4x less on the 1x-mode-capped DVE.
    #
    # Software-pipelined emission: group g's DVE row-sum is emitted after
    # group g+1's submax so the DVE never sits waiting on the ACT exps of
    # the group it just reduced. An accumulator tile every other group
    # (f=1/4 of tiles) offloads some row-sums from DVE to ScalarE.
    def group_lo_bank(g):
        return 1 if (act_accum and g % 2 == 0) else 0

    def emit_sum(pyp, pg):
        lb = group_lo_bank(pg)
        nc.vector.reduce_sum(
            out=s_sb[:, pg * BGRP + lb : (pg + 1) * BGRP],
            in_=pyp[:, lb:BGRP, 0:L2],
            axis=mybir.AxisListType.X,
        )

    # HAM warm-up: ~10 throwaway matmuls on the (tiny, early-arriving)
    # coefficient tile keep PE busy through the x-shard DMA wait, so the
    # clock gate reaches 2.4 GHz before the real matmul stream starts and
    # the DMA wait isn't dead PE time.
    warm = psum.tile([PART, BGRP, 512], mybir.dt.float32, tag="yp")
    for w in range(10):
        nc.tensor.matmul(
            warm[:, w % BGRP, 0:L2], ldhi_sb[:, 0:PART], ldhi_sb,
            start=True, stop=True,
        )

    pending_sum = None  # (yp of previous group, its group index)
    for g in range(N_GRPS):
        yp = psum.tile([PART, BGRP, 512], mybir.dt.float32, tag="yp")
        for i in range(BGRP):
            t = g * BGRP + i
            lhsT = xmt_sb[:, t * PART : (t + 1) * PART]
            nc.tensor.matmul(
                yp[:, i, 0:L2], lhsT, ldhi_sb, start=True, stop=False
            )
            nc.tensor.matmul(
                yp[:, i, 0:L2], lhsT, ldlo_sb, start=False, stop=True
            )
        if do_dve:
            # stride-8 submax: offline max gap to the true row max is 12.2
            # (overflow budget ~80), and it halves the DVE reduce cost
            nc.vector.reduce_max(
                out=negm_sb[:, g * BGRP : (g + 1) * BGRP],
                in_=yp[:, :, 2:L2:8],
                axis=mybir.AxisListType.X,
                negate=True,
            )
            if pending_sum is not None:
                emit_sum(*pending_sum)
        # exp in place in PSUM (PSUM src/dst has the smaller ScalarE bubble)
        if do_act:
            for i in range(BGRP):
                t = g * BGRP + i
                if i == 0 and act_accum and g % 2 == 0:
                    nc.scalar.activation(
                        out=yp[:, i, 0:L2],
                        in_=yp[:, i, 0:L2],
                        func=mybir.ActivationFunctionType.Exp,
                        bias=negm_sb[:, t : t + 1],
                        scale=1.0,
                        accum_out=s_sb[:, t : t + 1],
                    )
                else:
                    nc.scalar.activation(
                        out=yp[:, i, 0:L2],
                        in_=yp[:, i, 0:L2],
                        func=mybir.ActivationFunctionType.Exp,
                        bias=negm_sb[:, t : t + 1],
                        scale=1.0,
                    )
        pending_sum = (yp, g)
    if do_dve:
        emit_sum(*pending_sum)


def _build_module(reps=1, do_dve=True, do_act=True, act_accum=True):
    """Build + bacc-compile the module. reps>1 wraps the compute in a
    device-side loop (bench-only: wall-clock slope over the trip count
    cancels the large axon dispatch overhead)."""
    nc = bacc.Bacc("TRN2", target_bir_lowering=False, debug=False)
    xmt_d = nc.dram_tensor(
        "xmt", [K_DIM, ROWS_PER_CORE], mybir.dt.bfloat16, kind="ExternalInput"
    ).ap()
    ldhi_d = nc.dram_tensor(
        "ldhi", [K_DIM, L2], mybir.dt.bfloat16, kind="ExternalInput"
    ).ap()
    ldlo_d = nc.dram_tensor(
        "ldlo", [K_DIM, L2], mybir.dt.bfloat16, kind="ExternalInput"
    ).ap()
    s_d = nc.dram_tensor(
        "s_out", [PART, N_TILES], mybir.dt.float32, kind="ExternalOutput"
    ).ap()
    negm_d = nc.dram_tensor(
        "negm_out", [PART, N_TILES], mybir.dt.float32, kind="ExternalOutput"
    ).ap()

    with tile.TileContext(nc) as tc:
        with (
            tc.tile_pool(name="xpool", bufs=1) as xpool,
            tc.tile_pool(name="consts", bufs=1) as consts,
            tc.tile_pool(name="stats", bufs=1) as stats,
            tc.tile_pool(name="psum", bufs=4, space="PSUM") as psum,
        ):
            xmt_sb = xpool.tile([K_DIM, ROWS_PER_CORE], mybir.dt.bfloat16)
            ldhi_sb = consts.tile([K_DIM, L2], mybir.dt.bfloat16)
            ldlo_sb = consts.tile([K_DIM, L2], mybir.dt.bfloat16)
            negm_sb = stats.tile([PART, N_TILES], mybir.dt.float32)
            s_sb = stats.tile([PART, N_TILES], mybir.dt.float32)

            nc.sync.dma_start(out=ldhi_sb, in_=ldhi_d)
            nc.sync.dma_start(out=ldlo_sb, in_=ldlo_d)

            if not (do_dve and do_act):
                # ablation variants leave parts of the outputs unwritten;
                # initialize so the output DMAs have allocated sources
                nc.gpsimd.memset(s_sb, 1.0)
                nc.gpsimd.memset(negm_sb, 0.0)

            if reps == 1:
                _emit_compute(nc, tc, stats, psum, xmt_d, xmt_sb,
                              ldhi_sb, ldlo_sb, negm_sb, s_sb,
                              do_dve=do_dve, do_act=do_act, act_accum=act_accum)
            else:
                with tc.For_i(0, reps, 1, hint_engines=(mybir.EngineType.PE,)):
                    _emit_compute(nc, tc, stats, psum, xmt_d, xmt_sb,
                                  ldhi_sb, ldlo_sb, negm_sb, s_sb,
                                  do_dve=do_dve, do_act=do_act,
                                  act_accum=act_accum)

            nc.sync.dma_start(out=s_d, in_=s_sb)
            nc.sync.dma_start(out=negm_d, in_=negm_sb)

    nc.compile()
    return nc


def _compile():
    global _COMPILED
    if _COMPILED is None:
        _COMPILED = _build_module(reps=1)
    return _COMPILED


def _host_constants(W, b, perms, L):
    """LdAug (K_DIM, L2) float64: rows 0..m-1 = logits, row m = c_row."""
    perm = np.asarray(perms)[-1]
    idx = perm[:M_DIM]
    Wm = np.asarray(W, np.float64)[idx]
    bm = np.asarray(b, np.float64)[idx]

    zx = np.linspace(-5.0, 5.0, L)
    z1, z2 = np.meshgrid(zx, zx, indexing="xy")
    z_int = np.stack([z1.reshape(-1), z2.reshape(-1)], axis=1)  # (L2, 2)
    log_w = 2.0 * np.log(10.0 / L)
    log_p_z = -np.log(2.0 * np.pi) - 0.5 * np.sum(z_int**2, axis=1)

    logits = Wm @ z_int.T + bm[:, None]  # (m, L2)
    log_p0 = -np.logaddexp(0.0, logits)  # log sigmoid(-logits)
    c_row = log_w + log_p_z + log_p0.sum(axis=0)  # (L2,)
    return np.concatenate([logits, c_row[None, :]], axis=0), idx


def kernel(x, W, b, perms, bins):
    global LAST_RESULTS
    L = int(bins)
    assert L == L_BINS

    LdAug, idx = _host_constants(W, b, perms, L)
    hi = LdAug.astype(ml_dtypes.bfloat16)
    lo = (LdAug - hi.astype(np.float64)).astype(ml_dtypes.bfloat16)

    x_np = np.asarray(x, np.float32)
    assert x_np.shape == (N_OBS, D_DIM)
    xmt = np.empty((K_DIM, N_OBS), dtype=ml_dtypes.bfloat16)
    xmt[:M_DIM] = x_np[:, idx].T  # binary -> exact in bf16
    xmt[M_DIM] = 1.0

    nc = _compile()
    in_maps = []
    for c in range(N_CORES):
        shard = np.ascontiguousarray(
            xmt[:, c * ROWS_PER_CORE : (c + 1) * ROWS_PER_CORE]
        )
        in_maps.append({"xmt": shard, "ldhi": hi, "ldlo": lo})

    res = run_bass_kernel_spmd(nc, in_maps, core_ids=list(range(N_CORES)))
    LAST_RESULTS = res

    total = 0.0
    for c in range(N_CORES):
        s = res.results[c]["s_out"].astype(np.float64)
        mx = -res.results[c]["negm_out"].astype(np.float64)
        total += (np.log(s) + mx).sum()

    loss = -(D_DIM * total) / (N_PERM * M_DIM * N_OBS)
    return np.asarray(loss, dtype=np.float32)



# revision 22
# speedup vs baseline: 4375.6594x; 4375.6594x over previous
"""Bass/Tile TRN2 kernel for nn_BernoulliMaskedPPCA (loss_fn), v2.

Math (see reference): m = int(0.15*D) = 117 masked dims from the LAST
permutation only,
    li[r,c] = x_r . logits[:,c] + c_row[c]          (N, 400)
    loss = -(D / (P*m*N)) * sum_r logsumexp_c(li[r,c])

v2 exploits the loose tolerance (2e-2; this kernel lands ~1e-5):
  - Column pruning: the posterior mass lives in a small elliptical blob of
    the 20x20 z-grid. Keep the top C=64 columns by the x-independent score
    mean_c + 4*sd_c (Gaussian stats of li[.,c] from W, b and the column
    means of x). Validated: best dropped column sits >=9.9 below every
    row's lse (err ~1e-11 from pruning alone), stable under re-seeded x.
  - Global shift: row lse values span only [-119, -74], well inside the
    fp32/bf16 exp window, so a single constant shift s = mean_{c*} (folded
    into the constants row) replaces the per-row max. No DVE max pass, no
    max output, and the host adds N*s back analytically.
  - fp8 x: binary x is exact in e4m3; halves the HBM traffic (0.98 MB per
    core). Weights: single bf16 matmul (mixed operand dtypes) by default,
    with fp8 hi/lo two-matmul and all-bf16 fallbacks.
  - Constants row split over 3 ones-rows of the augmented x (K=120), so
    the per-column constant is represented to ~1e-3 even in fp8.
  - Whole shard's GEMM output (64 tiles x 64 cols f32) fills PSUM exactly
    once: tile t -> bank t//8, cols 64*(t%8). One start=True per bank
    (PSUM lazy-zero covers the packed neighbors), accumulate into the
    same bank region otherwise. No PSUM reuse, no WAR hazards.
  - Batched ScalarE exp (2 banks = 1024 els/instr, no bias, no accum) into
    SBUF bf16; batched DVE row-sums from SBUF bf16 (2x DVE mode) into a
    [128, 64] bf16 output. Host does log in f64.
"""

import numpy as np
import ml_dtypes

import concourse.bacc as bacc
import concourse.tile as tile
import concourse.mybir as mybir
from concourse.bass_utils import run_bass_kernel_spmd

N_CORES = 8
N_OBS = 65536
D_DIM = 784
M_DIM = 117          # int(784 * 0.15)
N_ONES = 3           # ones rows carrying the split constants
K_AUG = M_DIM + N_ONES  # 120
L_BINS = 20
N_PERM = 4
C_KEEP = 64          # pruned quadrature columns
ROWS_PER_CORE = N_OBS // N_CORES  # 8192
PART = 128
N_TILES = ROWS_PER_CORE // PART   # 64
TPB = 8              # tiles packed per PSUM bank (8*64 f32 = 2KB = 1 bank)
BANKS_PER_GRP = 2    # PSUM banks per ACT/DVE instruction group
N_GRPS = 4           # 4 groups x 2 banks x 8 tiles = 64 tiles
NB_TOT = N_GRPS * BANKS_PER_GRP  # 8 banks
N_CHUNKS = 4         # x-shard DMA chunks

WEIGHT_MODE = "mixed"   # mixed | fp8hilo | fp8single | bf16
N_WARM = 13

F8 = ml_dtypes.float8_e4m3
BF = ml_dtypes.bfloat16

_COMPILED = None
LAST_RESULTS = None


def _x_np_dtype():
    return BF if WEIGHT_MODE == "bf16" else F8


def _x_bir_dtype():
    return mybir.dt.bfloat16 if WEIGHT_MODE == "bf16" else mybir.dt.float8e4


def _w_bir_dtype():
    if WEIGHT_MODE in ("mixed", "bf16"):
        return mybir.dt.bfloat16
    return mybir.dt.float8e4


def _emit_compute(nc, tc, consts_sb, consts_d, stats, psum, exps, xmt_d,
                  xmt_sb, s_sb, s_d):
    ldhi_sb, ldlo_sb, warm_sb = consts_sb
    ldhi_d, ldlo_d = consts_d

    # Warm scratch memset first on the Pool queue (warmups wait on it).
    # The exp-table prime uses scale=0 (exp(0*garbage+0)=1) so it needs no
    # initialized input and the ~1.3us table load starts immediately.
    nc.gpsimd.memset(warm_sb, 0.0)
    prime = stats.tile([PART, 1], mybir.dt.float32, tag="prime")
    nc.scalar.activation(
        out=prime, in_=prime, func=mybir.ActivationFunctionType.Exp,
        scale=0.0,
    )

    # Split DMA dispatch across the SP and Pool queues: dispatch is ~500+ns
    # serial per queue and chunk k's completion must beat PE's arrival at
    # tile 16k (a blocked wait eats the ~1.7us DMA completion wakeup).
    # Consts go first on Pool (tiny), x chunks 0/1 on SP, 2/3 on Pool.
    nc.gpsimd.dma_start(out=ldhi_sb, in_=ldhi_d)
    if WEIGHT_MODE == "fp8hilo":
        nc.gpsimd.dma_start(out=ldlo_sb, in_=ldlo_d)
    chunk = ROWS_PER_CORE // N_CHUNKS
    for k in range(N_CHUNKS):
        sl = slice(k * chunk, (k + 1) * chunk)
        eng = nc.sync if k < N_CHUNKS // 2 else nc.gpsimd
        eng.dma_start(out=xmt_sb[:, sl], in_=xmt_d[:, sl])

    yp_grps = []
    for g in range(N_GRPS):
        yp_g = psum.tile([PART, BANKS_PER_GRP, 512], mybir.dt.float32,
                         tag="yp", name=f"yp{g}")
        yp_grps.append(yp_g)

    # Clock-ramp warmups from a Pool-memset scratch: no DMA dependency, so
    # they start ~immediately and keep PE busy until the first x chunk's
    # completion sem has fired (avoiding the blocked-wait wakeup) while
    # ramping the clock gate. They write the last bank's first tile slot,
    # which the real start=True matmul lazily re-zeroes.
    warm_out = yp_grps[-1][:, BANKS_PER_GRP - 1, 0:C_KEEP]
    for _ in range(N_WARM):
        nc.tensor.matmul(
            warm_out, warm_sb, warm_sb[:, 0:C_KEEP], start=True, stop=True,
            skip_group_check=True,
        )

    hilo = WEIGHT_MODE == "fp8hilo"
    for g in range(N_GRPS):
        yp = yp_grps[g]
        for bi in range(BANKS_PER_GRP):
            for j in range(TPB):
                t = (g * BANKS_PER_GRP + bi) * TPB + j
                lhsT = xmt_sb[:, t * PART : (t + 1) * PART]
                out = yp[:, bi, j * C_KEEP : (j + 1) * C_KEEP]
                nc.tensor.matmul(
                    out, lhsT, ldhi_sb,
                    start=(j == 0), stop=(j == TPB - 1) and not hilo,
                    skip_group_check=True,
                )
                if hilo:
                    nc.tensor.matmul(
                        out, lhsT, ldlo_sb,
                        start=False, stop=(j == TPB - 1),
                        skip_group_check=True,
                    )
        ex = exps.tile(
            [PART, BANKS_PER_GRP, TPB, C_KEEP], mybir.dt.bfloat16, tag="ex"
        )
        # First/last group: per-bank exp instrs — the first exp can start
        # half a group earlier, and the final reduce chain ends sooner.
        if g in (0, N_GRPS - 1):
            for bi in range(BANKS_PER_GRP):
                nc.scalar.activation(
                    out=ex[:, bi], in_=yp[:, bi, 0 : TPB * C_KEEP],
                    func=mybir.ActivationFunctionType.Exp,
                )
        else:
            nc.scalar.activation(
                out=ex, in_=yp[:, :, 0 : TPB * C_KEEP],
                func=mybir.ActivationFunctionType.Exp,
            )
        # Per-bank reduces (vs per-group) shrink the post-last-compute tail.
        # bf16 out: offline-validated at ~1e-5 final rel err even under
        # worst-case sequential bf16 accumulation.
        with nc.allow_low_precision(reason="bf16 row-sums validated offline"):
            for bi in range(BANKS_PER_GRP):
                bk = g * BANKS_PER_GRP + bi
                nc.vector.reduce_sum(
                    out=s_sb[:, bk * TPB : (bk + 1) * TPB],
                    in_=ex[:, bi],
                    axis=mybir.AxisListType.X,
                )
        if g == N_GRPS - 2:
            # early out-DMA for banks 0..5: its completion sem fires long
            # before the end-of-kernel drain checks it
            nc.sync.dma_start(out=s_d[:, : 48], in_=s_sb[:, : 48])
    nc.sync.dma_start(out=s_d[:, 48:], in_=s_sb[:, 48:])


def _build_module(reps=1):
    nc = bacc.Bacc("TRN2", target_bir_lowering=False, debug=False)
    xd = _x_bir_dtype()
    wd = _w_bir_dtype()
    xmt_d = nc.dram_tensor(
        "xmt", [K_AUG, ROWS_PER_CORE], xd, kind="ExternalInput"
    ).ap()
    ldhi_d = nc.dram_tensor(
        "ldhi", [K_AUG, C_KEEP], wd, kind="ExternalInput"
    ).ap()
    ldlo_d = nc.dram_tensor(
        "ldlo", [K_AUG, C_KEEP], wd, kind="ExternalInput"
    ).ap()
    s_d = nc.dram_tensor(
        "s_out", [PART, N_TILES], mybir.dt.bfloat16, kind="ExternalOutput"
    ).ap()

    with tile.TileContext(nc) as tc:
        with (
            tc.tile_pool(name="xpool", bufs=1) as xpool,
            tc.tile_pool(name="consts", bufs=1) as consts,
            tc.tile_pool(name="stats", bufs=1) as stats,
            tc.tile_pool(name="exps", bufs=N_GRPS) as exps,
            tc.tile_pool(name="psum", bufs=N_GRPS, space="PSUM") as psum,
        ):
            xmt_sb = xpool.tile([K_AUG, ROWS_PER_CORE], xd)
            ldhi_sb = consts.tile([K_AUG, C_KEEP], wd)
            ldlo_sb = consts.tile([K_AUG, C_KEEP], wd)
            warm_sb = consts.tile([K_AUG, PART], wd)
            s_sb = stats.tile([PART, N_TILES], mybir.dt.bfloat16)

            csb = (ldhi_sb, ldlo_sb, warm_sb)
            cd = (ldhi_d, ldlo_d)
            if reps == 1:
                _emit_compute(nc, tc, csb, cd, stats, psum, exps,
                              xmt_d, xmt_sb, s_sb, s_d)
            else:
                with tc.For_i(0, reps, 1, hint_engines=(mybir.EngineType.PE,)):
                    _emit_compute(nc, tc, csb, cd, stats, psum,
                                  exps, xmt_d, xmt_sb, s_sb, s_d)

    nc.compile()
    return nc


def _compile():
    global _COMPILED
    if _COMPILED is None:
        _COMPILED = _build_module(reps=1)
    return _COMPILED


def _split_const(cp, slots, dtype):
    """Greedy hi/lo split of the per-column constant over `slots` rows."""
    out = []
    r = cp.astype(np.float64)
    for _ in range(slots):
        q = r.astype(dtype)
        out.append(q)
        r = r - q.astype(np.float64)
    return out


def _host_constants(W, b, perms, L, xbar):
    """Pruned-column constants + global shift, all from W/b/xbar (f64)."""
    perm = np.asarray(perms)[-1]
    idx = perm[:M_DIM]
    Wm = np.asarray(W, np.float64)[idx]
    bm = np.asarray(b, np.float64)[idx]

    zx = np.linspace(-5.0, 5.0, L)
    z1, z2 = np.meshgrid(zx, zx, indexing="xy")
    z_int = np.stack([z1.reshape(-1), z2.reshape(-1)], axis=1)
    log_p_z = -np.log(2.0 * np.pi) - 0.5 * np.sum(z_int**2, axis=1)
    logits = Wm @ z_int.T + bm[:, None]                      # (117, 400)
    c_row = (2.0 * np.log(10.0 / L) + log_p_z
             - np.logaddexp(0.0, logits).sum(axis=0))        # (400,)

    mean_c = c_row + xbar @ logits
    sd_c = np.sqrt((xbar * (1.0 - xbar)) @ logits**2)
    score = mean_c + 4.0 * sd_c
    keep = np.sort(np.argsort(-score)[:C_KEEP])
    s_global = float(mean_c.max())

    lg = logits[:, keep]                                     # (117, C)
    cp = c_row[keep] - s_global                              # (C,)

    if WEIGHT_MODE == "fp8hilo":
        lhi = lg.astype(F8)
        llo = (lg - lhi.astype(np.float64)).astype(F8)
        cs = _split_const(cp, 2 * N_ONES, F8)
        hi = np.concatenate([lhi] + [c[None] for c in cs[0::2]], axis=0)
        lo = np.concatenate([llo] + [c[None] for c in cs[1::2]], axis=0)
        return idx, s_global, hi.astype(F8), lo.astype(F8)

    wdt = BF if WEIGHT_MODE in ("mixed", "bf16") else F8
    lq = lg.astype(wdt)
    cs = _split_const(cp, N_ONES, wdt)
    ld = np.concatenate([lq] + [c[None] for c in cs], axis=0)
    return idx, s_global, ld.astype(wdt), np.zeros_like(ld)


def kernel(x, W, b, perms, bins):
    global LAST_RESULTS
    L = int(bins)
    assert L == L_BINS

    x_np = np.asarray(x, np.float32)
    assert x_np.shape == (N_OBS, D_DIM)
    perm = np.asarray(perms)[-1]
    idx = perm[:M_DIM]
    xm_t = x_np[:, idx].T                       # (117, N) binary
    xbar = xm_t.mean(axis=1).astype(np.float64)

    idx2, s_global, hi, lo = _host_constants(W, b, perms, L, xbar)

    xdt = _x_np_dtype()
    xmt = np.empty((K_AUG, N_OBS), dtype=xdt)
    xmt[:M_DIM] = xm_t                          # binary -> exact in fp8/bf16
    xmt[M_DIM:] = 1.0

    nc = _compile()
    in_maps = []
    for c in range(N_CORES):
        shard = np.ascontiguousarray(
            xmt[:, c * ROWS_PER_CORE : (c + 1) * ROWS_PER_CORE]
        )
        in_maps.append({"xmt": shard, "ldhi": hi, "ldlo": lo})

    res = run_bass_kernel_spmd(nc, in_maps, core_ids=list(range(N_CORES)))
    LAST_RESULTS = res

    total = 0.0
    for c in range(N_CORES):
        s = res.results[c]["s_out"].astype(np.float64)
        total += np.log(s + 1e-30).sum()
    total += N_OBS * s_global

    loss = -(D_DIM * total) / (N_PERM * M_DIM * N_OBS)
    return np.asarray(loss, dtype=np.float32)


# revision 37
# speedup vs baseline: 46657.1726x; 10.6629x over previous
"""Bass/Tile TRN2 kernel for nn_BernoulliMaskedPPCA (loss_fn), v2.

Math (see reference): m = int(0.15*D) = 117 masked dims from the LAST
permutation only,
    li[r,c] = x_r . logits[:,c] + c_row[c]          (N, 400)
    loss = -(D / (P*m*N)) * sum_r logsumexp_c(li[r,c])

v2 exploits the loose tolerance (2e-2; this kernel lands ~1e-5):
  - Column pruning: the posterior mass lives in a small elliptical blob of
    the 20x20 z-grid. Keep the top C=64 columns by the x-independent score
    mean_c + 4*sd_c (Gaussian stats of li[.,c] from W, b and the column
    means of x). Validated: best dropped column sits >=9.9 below every
    row's lse (err ~1e-11 from pruning alone), stable under re-seeded x.
  - Global shift: row lse values span only [-119, -74], well inside the
    fp32/bf16 exp window, so a single constant shift s = mean_{c*} (folded
    into the constants row) replaces the per-row max. No DVE max pass, no
    max output, and the host adds N*s back analytically.
  - fp8 x: binary x is exact in e4m3; halves the HBM traffic (0.98 MB per
    core). Weights: single bf16 matmul (mixed operand dtypes) by default,
    with fp8 hi/lo two-matmul and all-bf16 fallbacks.
  - Constants row split over 3 ones-rows of the augmented x (K=120), so
    the per-column constant is represented to ~1e-3 even in fp8.
  - Whole shard's GEMM output (64 tiles x 64 cols f32) fills PSUM exactly
    once: tile t -> bank t//8, cols 64*(t%8). One start=True per bank
    (PSUM lazy-zero covers the packed neighbors), accumulate into the
    same bank region otherwise. No PSUM reuse, no WAR hazards.
  - Batched ScalarE exp (2 banks = 1024 els/instr, no bias, no accum) into
    SBUF bf16; batched DVE row-sums from SBUF bf16 (2x DVE mode) into a
    [128, 64] bf16 output. Host does log in f64.
"""

import numpy as np
import ml_dtypes

import concourse.bacc as bacc
import concourse.tile as tile
import concourse.mybir as mybir
from concourse.bass_utils import run_bass_kernel_spmd

N_CORES = 8
N_OBS = 65536
D_DIM = 784
M_DIM = 117          # int(784 * 0.15)
N_ONES = 3           # ones rows carrying the split constants
K_AUG = M_DIM + N_ONES  # 120
L_BINS = 20
N_PERM = 4
C_KEEP = 64          # pruned quadrature columns
ROWS_PER_CORE = N_OBS // N_CORES  # 8192
PART = 128
N_TILES = ROWS_PER_CORE // PART   # 64
TPB = 8              # tiles packed per PSUM bank (8*64 f32 = 2KB = 1 bank)
BANKS_PER_GRP = 2    # PSUM banks per ACT/DVE instruction group
N_GRPS = 4           # 4 groups x 2 banks x 8 tiles = 64 tiles
NB_TOT = N_GRPS * BANKS_PER_GRP  # 8 banks
import os as _os

N_CHUNKS = int(_os.environ.get("KCHUNKS", 8))   # x-shard DMA chunks
N_SP = int(_os.environ.get("KSP", 2))           # chunks on the SP queue

WEIGHT_MODE = _os.environ.get("KWMODE", "mixed")
N_WARM = int(_os.environ.get("KWARM", 13))

F8 = ml_dtypes.float8_e4m3
BF = ml_dtypes.bfloat16

_COMPILED = None
LAST_RESULTS = None


def _x_np_dtype():
    return BF if WEIGHT_MODE == "bf16" else F8


def _x_bir_dtype():
    return mybir.dt.bfloat16 if WEIGHT_MODE == "bf16" else mybir.dt.float8e4


def _w_bir_dtype():
    if WEIGHT_MODE in ("mixed", "bf16"):
        return mybir.dt.bfloat16
    return mybir.dt.float8e4


def _emit_prologue(nc, tc, consts_sb, consts_d, stats, psum, s_sb,
                   do_pe=True, do_dve=True):
    """Loop-invariant work, emitted once before the (optional) reps loop:
    const DMAs, warm-scratch memset, exp-table prime, clock-ramp warmups."""
    ldhi_sb, ldlo_sb, warm_sb = consts_sb
    ldhi_d, ldlo_d = consts_d

    # Warm scratch memset first on the Pool queue (warmups wait on it).
    # The exp-table prime uses scale=0 (exp(0*garbage+0)=1) so it needs no
    # initialized input and the ~1.3us table load starts immediately.
    nc.gpsimd.memset(warm_sb, 0.0)
    if not do_dve:
        # ablation variants leave s_sb unwritten; give the out-DMA a source
        nc.gpsimd.memset(s_sb, 1.0)
    prime = stats.tile([PART, 1], mybir.dt.float32, tag="prime")
    nc.scalar.activation(
        out=prime, in_=prime, func=mybir.ActivationFunctionType.Exp,
        scale=0.0,
    )
    nc.gpsimd.dma_start(out=ldhi_sb, in_=ldhi_d)
    if WEIGHT_MODE == "fp8hilo":
        nc.gpsimd.dma_start(out=ldlo_sb, in_=ldlo_d)

    # Clock-ramp warmups from the memset scratch: no DMA dependency, so
    # they start ~immediately and keep PE busy until the first x chunk's
    # completion sem has fired (avoiding the blocked-wait wakeup) while
    # ramping the clock gate. They write into a scratch PSUM tile that the
    # first real start=True matmul in that bank lazily re-zeroes.
    if do_pe:
        warm_yp = psum.tile([PART, BANKS_PER_GRP, 512], mybir.dt.float32,
                            tag="yp", name="warm_yp")
        for _ in range(N_WARM):
            nc.tensor.matmul(
                warm_yp[:, BANKS_PER_GRP - 1, 0:C_KEEP], warm_sb,
                warm_sb[:, 0:C_KEEP], start=True,
                stop=True, skip_group_check=True,
            )


def _emit_compute(nc, tc, consts_sb, consts_d, stats, psum, exps, xpool,
                  xmt_d, s_sb, s_d, do_pe=True, do_act=True, do_dve=True,
                  do_xdma=True):
    # do_pe/do_act/do_dve are bench-only ablation switches (numerically
    # wrong when False; used to attribute HW time per engine).
    ldhi_sb, ldlo_sb, warm_sb = consts_sb

    # The x shard is double-buffered (xpool bufs=2): in the bench reps
    # loop, iteration i+1's chunk DMAs overlap iteration i's compute.
    xmt_sb = xpool.tile([K_AUG, ROWS_PER_CORE], _x_bir_dtype(), tag="xmt")

    # Split DMA dispatch across the SP and Pool queues: dispatch is ~500+ns
    # serial per queue and chunk k's completion must beat PE's arrival at
    # tile 16k (a blocked wait eats the ~1.7us DMA completion wakeup).
    chunk = ROWS_PER_CORE // N_CHUNKS
    if do_xdma:
        for k in range(N_CHUNKS):
            sl = slice(k * chunk, (k + 1) * chunk)
            eng = nc.sync if k < N_SP else nc.gpsimd
            eng.dma_start(out=xmt_sb[:, sl], in_=xmt_d[:, sl])
    elif do_pe:
        nc.gpsimd.memset(xmt_sb[:, 0:PART], 0.0)

    yp_grps = []
    for g in range(N_GRPS):
        yp_g = psum.tile([PART, BANKS_PER_GRP, 512], mybir.dt.float32,
                         tag="yp", name=f"yp{g}")
        yp_grps.append(yp_g)

    hilo = WEIGHT_MODE == "fp8hilo"
    for g in range(N_GRPS):
        yp = yp_grps[g]
        if not do_pe:
            # ablation: one tiny matmul per bank allocates/zeroes it so the
            # ACT/DVE stages have a valid source
            for bi in range(BANKS_PER_GRP):
                nc.tensor.matmul(
                    yp[:, bi, 0:C_KEEP], warm_sb, warm_sb[:, 0:C_KEEP],
                    start=True, stop=True, skip_group_check=True,
                )
        if do_pe:
            for bi in range(BANKS_PER_GRP):
                for j in range(TPB):
                    t = (g * BANKS_PER_GRP + bi) * TPB + j
                    lhsT = xmt_sb[:, t * PART : (t + 1) * PART]
                    out = yp[:, bi, j * C_KEEP : (j + 1) * C_KEEP]
                    nc.tensor.matmul(
                        out, lhsT, ldhi_sb,
                        start=(j == 0), stop=(j == TPB - 1) and not hilo,
                        skip_group_check=True,
                    )
                    if hilo:
                        nc.tensor.matmul(
                            out, lhsT, ldlo_sb,
                            start=False, stop=(j == TPB - 1),
                            skip_group_check=True,
                        )
        ex = exps.tile(
            [PART, BANKS_PER_GRP, TPB, C_KEEP], mybir.dt.bfloat16, tag="ex"
        )
        # First/last group: per-bank exp instrs — the first exp can start
        # half a group earlier, and the final reduce chain ends sooner.
        if do_act:
            if g in (0, N_GRPS - 1):
                for bi in range(BANKS_PER_GRP):
                    nc.scalar.activation(
                        out=ex[:, bi], in_=yp[:, bi, 0 : TPB * C_KEEP],
                        func=mybir.ActivationFunctionType.Exp,
                    )
            else:
                nc.scalar.activation(
                    out=ex, in_=yp[:, :, 0 : TPB * C_KEEP],
                    func=mybir.ActivationFunctionType.Exp,
                )
        elif do_dve:
            nc.vector.memset(ex, 1.0)
        # Per-bank reduces (vs per-group) shrink the post-last-compute tail.
        # bf16 out: offline-validated at ~1e-5 final rel err even under
        # worst-case sequential bf16 accumulation.
        if do_dve:
            with nc.allow_low_precision(
                reason="bf16 row-sums validated offline"
            ):
                for bi in range(BANKS_PER_GRP):
                    bk = g * BANKS_PER_GRP + bi
                    nc.vector.reduce_sum(
                        out=s_sb[:, bk * TPB : (bk + 1) * TPB],
                        in_=ex[:, bi],
                        axis=mybir.AxisListType.X,
                    )
        if g == N_GRPS - 2 and do_dve:
            # early out-DMA for banks 0..5: its completion sem fires long
            # before the end-of-kernel drain checks it
            nc.sync.dma_start(out=s_d[:, : 48], in_=s_sb[:, : 48])
    nc.sync.dma_start(out=s_d[:, 48:], in_=s_sb[:, 48:])


def _build_module(reps=1, do_pe=True, do_act=True, do_dve=True,
                  do_xdma=True):
    nc = bacc.Bacc("TRN2", target_bir_lowering=False, debug=False)
    xd = _x_bir_dtype()
    wd = _w_bir_dtype()
    xmt_d = nc.dram_tensor(
        "xmt", [K_AUG, ROWS_PER_CORE], xd, kind="ExternalInput"
    ).ap()
    ldhi_d = nc.dram_tensor(
        "ldhi", [K_AUG, C_KEEP], wd, kind="ExternalInput"
    ).ap()
    ldlo_d = nc.dram_tensor(
        "ldlo", [K_AUG, C_KEEP], wd, kind="ExternalInput"
    ).ap()
    s_d = nc.dram_tensor(
        "s_out", [PART, N_TILES], mybir.dt.bfloat16, kind="ExternalOutput"
    ).ap()

    with tile.TileContext(nc) as tc:
        with (
            tc.tile_pool(name="xpool", bufs=2) as xpool,
            tc.tile_pool(name="consts", bufs=1) as consts,
            tc.tile_pool(name="stats", bufs=1) as stats,
            tc.tile_pool(name="exps", bufs=N_GRPS) as exps,
            tc.tile_pool(name="psum", bufs=N_GRPS, space="PSUM") as psum,
        ):
            ldhi_sb = consts.tile([K_AUG, C_KEEP], wd)
            ldlo_sb = consts.tile([K_AUG, C_KEEP], wd)
            warm_sb = consts.tile([K_AUG, PART], wd)
            s_sb = stats.tile([PART, N_TILES], mybir.dt.bfloat16)

            csb = (ldhi_sb, ldlo_sb, warm_sb)
            cd = (ldhi_d, ldlo_d)
            kw = dict(do_pe=do_pe, do_act=do_act, do_dve=do_dve,
                      do_xdma=do_xdma)
            _emit_prologue(nc, tc, csb, cd, stats, psum, s_sb,
                           do_pe=do_pe, do_dve=do_dve)
            if reps == 1:
                _emit_compute(nc, tc, csb, cd, stats, psum, exps,
                              xpool, xmt_d, s_sb, s_d, **kw)
            else:
                with tc.For_i(0, reps, 1, hint_engines=(mybir.EngineType.PE,)):
                    _emit_compute(nc, tc, csb, cd, stats, psum,
                                  exps, xpool, xmt_d, s_sb, s_d, **kw)

    nc.compile()
    return nc


def _compile():
    global _COMPILED
    if _COMPILED is None:
        _COMPILED = _build_module(reps=1)
    return _COMPILED


def _split_const(cp, slots, dtype):
    """Greedy hi/lo split of the per-column constant over `slots` rows."""
    out = []
    r = cp.astype(np.float64)
    for _ in range(slots):
        q = r.astype(dtype)
        out.append(q)
        r = r - q.astype(np.float64)
    return out


def _host_constants(W, b, perms, L, xbar):
    """Pruned-column constants + global shift, all from W/b/xbar (f64)."""
    perm = np.asarray(perms)[-1]
    idx = perm[:M_DIM]
    Wm = np.asarray(W, np.float64)[idx]
    bm = np.asarray(b, np.float64)[idx]

    zx = np.linspace(-5.0, 5.0, L)
    z1, z2 = np.meshgrid(zx, zx, indexing="xy")
    z_int = np.stack([z1.reshape(-1), z2.reshape(-1)], axis=1)
    log_p_z = -np.log(2.0 * np.pi) - 0.5 * np.sum(z_int**2, axis=1)
    logits = Wm @ z_int.T + bm[:, None]                      # (117, 400)
    c_row = (2.0 * np.log(10.0 / L) + log_p_z
             - np.logaddexp(0.0, logits).sum(axis=0))        # (400,)

    mean_c = c_row + xbar @ logits
    sd_c = np.sqrt((xbar * (1.0 - xbar)) @ logits**2)
    score = mean_c + 4.0 * sd_c
    keep = np.sort(np.argsort(-score)[:C_KEEP])
    s_global = float(mean_c.max())

    lg = logits[:, keep]                                     # (117, C)
    cp = c_row[keep] - s_global                              # (C,)

    if WEIGHT_MODE == "fp8hilo":
        lhi = lg.astype(F8)
        llo = (lg - lhi.astype(np.float64)).astype(F8)
        cs = _split_const(cp, 2 * N_ONES, F8)
        hi = np.concatenate([lhi] + [c[None] for c in cs[0::2]], axis=0)
        lo = np.concatenate([llo] + [c[None] for c in cs[1::2]], axis=0)
        return idx, s_global, hi.astype(F8), lo.astype(F8)

    wdt = BF if WEIGHT_MODE in ("mixed", "bf16") else F8
    lq = lg.astype(wdt)
    cs = _split_const(cp, N_ONES, wdt)
    ld = np.concatenate([lq] + [c[None] for c in cs], axis=0)
    return idx, s_global, ld.astype(wdt), np.zeros_like(ld)


def kernel(x, W, b, perms, bins):
    global LAST_RESULTS
    L = int(bins)
    assert L == L_BINS

    x_np = np.asarray(x, np.float32)
    assert x_np.shape == (N_OBS, D_DIM)
    perm = np.asarray(perms)[-1]
    idx = perm[:M_DIM]
    xm_t = x_np[:, idx].T                       # (117, N) binary
    xbar = xm_t.mean(axis=1).astype(np.float64)

    idx2, s_global, hi, lo = _host_constants(W, b, perms, L, xbar)

    xdt = _x_np_dtype()
    xmt = np.empty((K_AUG, N_OBS), dtype=xdt)
    xmt[:M_DIM] = xm_t                          # binary -> exact in fp8/bf16
    xmt[M_DIM:] = 1.0

    nc = _compile()
    in_maps = []
    for c in range(N_CORES):
        shard = np.ascontiguousarray(
            xmt[:, c * ROWS_PER_CORE : (c + 1) * ROWS_PER_CORE]
        )
        in_maps.append({"xmt": shard, "ldhi": hi, "ldlo": lo})

    res = run_bass_kernel_spmd(nc, in_maps, core_ids=list(range(N_CORES)))
    LAST_RESULTS = res

    total = 0.0
    for c in range(N_CORES):
        s = res.results[c]["s_out"].astype(np.float64)
        total += np.log(s + 1e-30).sum()
    total += N_OBS * s_global

    loss = -(D_DIM * total) / (N_PERM * M_DIM * N_OBS)
    return np.asarray(loss, dtype=np.float32)
